# revision 17
# baseline (speedup 1.0000x reference)
"""Trainium2 Bass kernel for a masked-attention block (MAB).

Computation (per batch element, all fp32):
    Q = X@Wq + bq ; K = Y@Wk + bk ; V = Y@Wv + bv
    logits = per-head Qh@Kh^T / 32, masked keys -> -inf, softmax over keys
    attn   = A @ Vh (concat heads)
    O1 = LN(Q + attn; g1,b1)
    O  = LN(O1 + relu(O1@Wo + bo); g2,b2)

Sharding: pure data-parallel, one batch element per NeuronCore (B=8 = 8 cores).

On-device dataflow is "feature-major": activations live in SBUF transposed
([model_dim -> 8x128 partitions, token -> free]).  With weights in natural
layout every matmul chains without any transposes:
    actT_out[n, t] = sum_d W[d, n] * actT_in[d, t]   (lhsT=W, rhs=actT_in)
Attention also chains: logitsT[k, q] from (lhsT=KT_h, rhs=QT_h) single
128-contraction; exp on ACT (mask folded in as a per-partition bias);
AV from (lhsT=V_natural, rhs=expT).  The softmax denominator and the
LayerNorm stats are partition-dim reductions done with all-ones stationary
matmuls (which also broadcast the result across partitions for free).
All matmuls use float32r (FP22 truncation) which runs at full PE rate for
moving free-dim >= 256.

The host transposes X/Y on the way in and the output on the way out, and
converts the bool mask into an additive f32 bias (0 / -1e4).
"""

import math
import numpy as np
from contextlib import ExitStack

import concourse.bass as bass
import concourse.mybir as mybir
import concourse.tile as tile
from concourse import bacc
from concourse.bass_utils import run_bass_kernel_spmd

P = 128
NX = 1024
NY = 1024
DIM = 1024
H = 8
KO = DIM // P          # 8 partition sub-tiles of the model dim
QC = 512               # moving-operand chunk (fp32 max free dim)
NQC = NX // QC         # 2
F32 = mybir.dt.float32
F32R = mybir.dt.float32r
AF = mybir.ActivationFunctionType
ALU = mybir.AluOpType
SCALE = 1.0 / 32.0     # 1/sqrt(DIM)
EPS = 1e-5


def _r(ap):
    return ap.bitcast(F32R)


def _build():
    nc = bacc.Bacc("TRN2", target_bir_lowering=False, debug=False,
                   enable_asserts=False)

    # ---- DRAM I/O (per-core shapes) ----
    XT = nc.dram_tensor("XT", [DIM, NX], F32, kind="ExternalInput").ap()
    YT = nc.dram_tensor("YT", [DIM, NY], F32, kind="ExternalInput").ap()
    MB = nc.dram_tensor("MB", [NY], F32, kind="ExternalInput").ap()
    Wd = {}
    for w in ("Wq", "Wk", "Wv", "Wo"):
        Wd[w] = nc.dram_tensor(w, [DIM, DIM], F32, kind="ExternalInput").ap()
    Vecs = {}
    for vname in ("bq", "bk", "bv", "bo", "g1", "b1", "g2", "b2"):
        Vecs[vname] = nc.dram_tensor(vname, [DIM], F32, kind="ExternalInput").ap()
    OT = nc.dram_tensor("OT", [DIM, NX], F32, kind="ExternalOutput").ap()

    xt3 = XT.rearrange("(ko p) q -> p ko q", p=P)
    yt3 = YT.rearrange("(ko p) q -> p ko q", p=P)
    wq3 = Wd["Wq"].rearrange("(ko p) d -> p ko d", p=P)
    wk3 = Wd["Wk"].rearrange("(ko p) d -> p ko d", p=P)
    wv3 = Wd["Wv"].rearrange("(ko p) d -> p ko d", p=P)
    wo3 = Wd["Wo"].rearrange("(ko p) d -> p ko d", p=P)
    ot3 = OT.rearrange("(do p) q -> p do q", p=P)

    with tile.TileContext(nc) as tc:
        with ExitStack() as octx:
            const = octx.enter_context(tc.tile_pool(name="const", bufs=1))
            actp = octx.enter_context(tc.tile_pool(name="act", bufs=3))

            # ---- constants ----
            # walrus requires every writer of an fp32r-matmul operand to have
            # an fp32r-tagged output AP; memset can't write f32r, so round
            # the ones through a copy
            ones128 = const.tile([P, P], F32, tag="ones", name="ones128")
            ones_tmp = const.tile([P, P], F32, tag="onest", name="ones_tmp")
            nc.vector.memset(ones_tmp, 1.0)
            nc.vector.tensor_copy(_r(ones128), ones_tmp)
            eps_sb = const.tile([P, 1], F32, tag="eps", name="eps_sb")
            nc.vector.memset(eps_sb, EPS)

            def vec_pko(name):
                t = const.tile([P, KO], F32, tag=f"v_{name}", name=f"{name}_sb")
                nc.sync.dma_start(t, Vecs[name].rearrange("(ko p) -> p ko", p=P))
                return t

            mb_sb = const.tile([P, KO], F32, tag="v_mb", name="mb_sb")
            nc.sync.dma_start(mb_sb, MB.rearrange("(ko p) -> p ko", p=P))
            bq_sb = vec_pko("bq")
            bk_sb = vec_pko("bk")
            bo_sb = vec_pko("bo")
            g1_sb = vec_pko("g1")
            b1_sb = vec_pko("b1")
            g2_sb = vec_pko("g2")
            b2_sb = vec_pko("b2")
            bv_sb = const.tile([1, DIM], F32, tag="v_bv", name="bv_sb")
            nc.sync.dma_start(_r(bv_sb),
                              _r(Vecs["bv"].rearrange("(one n) -> one n", one=1)))

            # ---- big feature-major activation tiles (rotating slots) ----
            qt = actp.tile([P, KO, NX], F32, tag="big", name="qt")
            ktm = actp.tile([P, KO, NY], F32, tag="big", name="ktm")
            vm = actp.tile([P, KO, DIM], F32, tag="big", name="vm")

            # ================= Phase 1: Q, K, V projections =================
            with tc.tile_pool(name="io", bufs=1) as iop, \
                 tc.tile_pool(name="w1", bufs=2) as wp, \
                 tc.tile_pool(name="gp1", bufs=8, space="PSUM") as pp:
                xt = iop.tile([P, KO, NX], F32, tag="xt", name="xt")
                yt = iop.tile([P, KO, NY], F32, tag="yt", name="yt")
                for k in range(KO):
                    nc.sync.dma_start(_r(xt[:, k, :]), _r(xt3[:, k, :]))
                for k in range(KO):
                    nc.sync.dma_start(_r(yt[:, k, :]), _r(yt3[:, k, :]))

                def proj_featmajor(w3, rhs_sb, out_sb, bias_sb, label):
                    # out_sb[p, do, q] (+= bias[do*128+p]) = sum_k W[k, d] rhs[k, q]
                    for dg in range(2):
                        wt = wp.tile([P, KO, QC], F32, tag="w", name=f"w_{label}{dg}")
                        for k in range(KO):
                            nc.sync.dma_start(_r(wt[:, k, :]),
                                              _r(w3[:, k, dg * QC:(dg + 1) * QC]))
                        for qc in range(NQC):
                            qs = slice(qc * QC, (qc + 1) * QC)
                            for d4 in range(4):
                                ps = pp.tile([P, QC], F32, tag="ps",
                                             name=f"ps_{label}{dg}{qc}{d4}")
                                for k in range(KO):
                                    nc.tensor.matmul(
                                        ps,
                                        lhsT=_r(wt[:, k, d4 * P:(d4 + 1) * P]),
                                        rhs=_r(rhs_sb[:, k, qs]),
                                        start=(k == 0), stop=(k == KO - 1))
                                do = dg * 4 + d4
                                nc.scalar.activation(
                                    _r(out_sb[:, do, qs]), ps, AF.Identity,
                                    bias=bias_sb[:, do:do + 1], scale=1.0)

                proj_featmajor(wq3, xt, qt, bq_sb, "q")
                proj_featmajor(wk3, yt, ktm, bk_sb, "k")

                # V in natural (token-major) layout: V[y, n] = sum_k Y[y,k] Wv[k,n]
                for ng in range(2):
                    wt = wp.tile([P, KO, QC], F32, tag="w", name=f"w_v{ng}")
                    for k in range(KO):
                        nc.sync.dma_start(_r(wt[:, k, :]),
                                          _r(wv3[:, k, ng * QC:(ng + 1) * QC]))
                    ns = slice(ng * QC, (ng + 1) * QC)
                    for yo in range(KO):
                        ps = pp.tile([P, QC], F32, tag="ps", name=f"ps_v{ng}{yo}")
                        for k in range(KO):
                            nc.tensor.matmul(
                                ps,
                                lhsT=_r(yt[:, k, yo * P:(yo + 1) * P]),
                                rhs=_r(wt[:, k, :]),
                                start=(k == 0), stop=False)
                        # fold per-free-dim bias bv with a K=1 ones matmul
                        nc.tensor.matmul(
                            ps, lhsT=_r(ones128[0:1, :]), rhs=_r(bv_sb[:, ns]),
                            start=False, stop=True)
                        nc.scalar.copy(_r(vm[:, yo, ns]), ps)

            # ================= Phase 2: attention =================
            with tc.tile_pool(name="zp", bufs=1) as zp:
                zt = zp.tile([P, KO, NX], F32, tag="z", name="zt")

                with tc.tile_pool(name="exp", bufs=9) as ep, \
                     tc.tile_pool(name="tree", bufs=2) as trp, \
                     tc.tile_pool(name="rcp", bufs=2) as rp, \
                     tc.tile_pool(name="lgp", bufs=3, space="PSUM") as lgp, \
                     tc.tile_pool(name="avp", bufs=2, space="PSUM") as avp, \
                     tc.tile_pool(name="rlp", bufs=2, space="PSUM") as rlp:
                    for h in range(H):
                        et = [ep.tile([P, NY], F32, tag="exp", name=f"et{h}_{k}")
                              for k in range(KO)]
                        # logitsT[k, q] = sum_d KT_h[d, k] QT_h[d, q]; exp with
                        # mask bias per key (partition) and 1/32 scale
                        for kt in range(KO):
                            for qc in range(NQC):
                                qs = slice(qc * QC, (qc + 1) * QC)
                                pl = lgp.tile([P, QC], F32, tag="lg",
                                              name=f"pl{h}{kt}{qc}")
                                nc.tensor.matmul(
                                    pl,
                                    lhsT=_r(ktm[:, h, kt * P:(kt + 1) * P]),
                                    rhs=_r(qt[:, h, qs]),
                                    start=True, stop=True)
                                nc.scalar.activation(
                                    _r(et[kt][:, qs]), pl, AF.Exp,
                                    bias=mb_sb[:, kt:kt + 1], scale=SCALE)
                        # softmax denominator: free-position-wise sum of the 8
                        # key sub-tiles on DVE, then an all-ones matmul that
                        # both reduces over partitions and broadcasts back
                        ta = trp.tile([P, NY], F32, tag="tr", name=f"ta{h}")
                        tb = trp.tile([P, NY], F32, tag="tr", name=f"tb{h}")
                        nc.vector.tensor_add(_r(ta), et[0], et[1])
                        nc.vector.tensor_add(tb, et[2], et[3])
                        nc.vector.tensor_add(_r(ta), ta, tb)
                        nc.vector.tensor_add(tb, et[4], et[5])
                        nc.vector.tensor_add(_r(ta), ta, tb)
                        nc.vector.tensor_add(tb, et[6], et[7])
                        nc.vector.tensor_add(_r(ta), ta, tb)
                        rc = rp.tile([P, NX], F32, tag="rc", name=f"rc{h}")
                        for qc in range(NQC):
                            qs = slice(qc * QC, (qc + 1) * QC)
                            pr = rlp.tile([P, QC], F32, tag="rl", name=f"pr{h}{qc}")
                            nc.tensor.matmul(pr, lhsT=_r(ones128), rhs=_r(ta[:, qs]),
                                             start=True, stop=True)
                            nc.vector.reciprocal(rc[:, qs], pr)
                        # attnT_h[d, q] = sum_k V[k, d_h] expT[k, q]; then
                        # normalize by the softmax denom and add the Q residual
                        for qc in range(NQC):
                            qs = slice(qc * QC, (qc + 1) * QC)
                            pa = avp.tile([P, QC], F32, tag="av", name=f"pa{h}{qc}")
                            for kt in range(KO):
                                nc.tensor.matmul(
                                    pa,
                                    lhsT=_r(vm[:, kt, h * P:(h + 1) * P]),
                                    rhs=_r(et[kt][:, qs]),
                                    start=(kt == 0), stop=(kt == KO - 1))
                            nc.vector.tensor_mul(_r(zt[:, h, qs]), pa, rc[:, qs])
                            nc.vector.tensor_add(_r(zt[:, h, qs]), zt[:, h, qs],
                                                 qt[:, h, qs])

                # ---- LayerNorm over the model dim (partition direction) ----
                def layernorm(in_sb, sqp, stp, spp, emit_out):
                    for qc in range(NQC):
                        qs = slice(qc * QC, (qc + 1) * QC)
                        pmu = spp.tile([P, QC], F32, tag="pmu", name=f"pmu{qc}")
                        ps2 = spp.tile([P, QC], F32, tag="ps2", name=f"ps2{qc}")
                        for do in range(KO):
                            nc.tensor.matmul(pmu, lhsT=_r(ones128),
                                             rhs=_r(in_sb[:, do, qs]),
                                             start=(do == 0), stop=(do == KO - 1))
                        for do in range(KO):
                            sq = sqp.tile([P, QC], F32, tag="sq", name=f"sq{qc}{do}")
                            nc.scalar.activation(_r(sq), in_sb[:, do, qs], AF.Square,
                                                 bias=0.0, scale=1.0)
                            nc.tensor.matmul(ps2, lhsT=_r(ones128), rhs=_r(sq),
                                             start=(do == 0), stop=(do == KO - 1))
                        mu = stp.tile([P, QC], F32, tag="mu", name=f"mu{qc}")
                        nc.vector.tensor_scalar_mul(mu, pmu, 1.0 / DIM)
                        msq = stp.tile([P, QC], F32, tag="msq", name=f"msq{qc}")
                        nc.vector.tensor_mul(msq, mu, mu)
                        sd = stp.tile([P, QC], F32, tag="sd", name=f"sd{qc}")
                        nc.vector.scalar_tensor_tensor(
                            sd, ps2, 1.0 / DIM, msq,
                            op0=ALU.mult, op1=ALU.subtract)
                        nc.scalar.activation(sd, sd, AF.Sqrt, bias=eps_sb, scale=1.0)
                        rsig = stp.tile([P, QC], F32, tag="rsig", name=f"rsig{qc}")
                        nc.vector.reciprocal(rsig, sd)
                        mrs = stp.tile([P, QC], F32, tag="mrs", name=f"mrs{qc}")
                        nc.vector.tensor_mul(mrs, mu, rsig)
                        for do in range(KO):
                            t = sqp.tile([P, QC], F32, tag="t", name=f"t{qc}{do}")
                            nc.vector.tensor_mul(t, in_sb[:, do, qs], rsig)
                            nc.vector.tensor_sub(t, t, mrs)
                            emit_out(do, qs, t)

                # LN1 -> o1t (feature-major)
                with tc.tile_pool(name="sq1", bufs=3) as sqp1, \
                     tc.tile_pool(name="st1", bufs=2) as stp1, \
                     tc.tile_pool(name="sp1", bufs=2, space="PSUM") as spp1:
                    o1t = actp.tile([P, KO, NX], F32, tag="big", name="o1t")

                    def emit_o1(do, qs, t):
                        nc.scalar.activation(_r(o1t[:, do, qs]), t, AF.Identity,
                                             bias=b1_sb[:, do:do + 1],
                                             scale=g1_sb[:, do:do + 1])

                    layernorm(zt, sqp1, stp1, spp1, emit_o1)

            # ================= Phase 3: output proj + LN2 =================
            with tc.tile_pool(name="w3", bufs=2) as wp3, \
                 tc.tile_pool(name="sq2", bufs=4) as sqp2, \
                 tc.tile_pool(name="st2", bufs=2) as stp2, \
                 tc.tile_pool(name="out", bufs=4) as outp, \
                 tc.tile_pool(name="gp3", bufs=4, space="PSUM") as pp3, \
                 tc.tile_pool(name="sp2", bufs=2, space="PSUM") as spp2:
                z2t = actp.tile([P, KO, NX], F32, tag="big", name="z2t")
                # HT[n, q] = sum_d Wo[d, n] O1T[d, q];  z2 = o1 + relu(H + bo)
                for ng in range(2):
                    wt = wp3.tile([P, KO, QC], F32, tag="w", name=f"w_o{ng}")
                    for k in range(KO):
                        nc.sync.dma_start(_r(wt[:, k, :]),
                                          _r(wo3[:, k, ng * QC:(ng + 1) * QC]))
                    for qc in range(NQC):
                        qs = slice(qc * QC, (qc + 1) * QC)
                        for n4 in range(4):
                            ps = pp3.tile([P, QC], F32, tag="ps",
                                          name=f"ps_o{ng}{qc}{n4}")
                            for k in range(KO):
                                nc.tensor.matmul(
                                    ps,
                                    lhsT=_r(wt[:, k, n4 * P:(n4 + 1) * P]),
                                    rhs=_r(o1t[:, k, qs]),
                                    start=(k == 0), stop=(k == KO - 1))
                            no = ng * 4 + n4
                            ht = sqp2.tile([P, QC], F32, tag="ht",
                                           name=f"ht{ng}{qc}{n4}")
                            nc.scalar.activation(ht, ps, AF.Relu,
                                                 bias=bo_sb[:, no:no + 1], scale=1.0)
                            nc.vector.tensor_add(_r(z2t[:, no, qs]), ht,
                                                 o1t[:, no, qs])

                def emit_o2(do, qs, t):
                    o = outp.tile([P, QC], F32, tag="o", name=f"o{do}")
                    nc.scalar.activation(o, t, AF.Identity,
                                         bias=b2_sb[:, do:do + 1],
                                         scale=g2_sb[:, do:do + 1])
                    nc.sync.dma_start(ot3[:, do, qs], o)

                layernorm(z2t, sqp2, stp2, spp2, emit_o2)

    nc.compile()
    return nc


_CACHE = {}


def _get_nc():
    if "nc" not in _CACHE:
        _CACHE["nc"] = _build()
    return _CACHE["nc"]


def make_in_maps(X, Y, mask, Wq, bq, Wk, bk, Wv, bv, Wo, bo, g1, b1, g2, b2):
    f = lambda a: np.ascontiguousarray(np.asarray(a, dtype=np.float32))
    shared = {
        "Wq": f(Wq), "Wk": f(Wk), "Wv": f(Wv), "Wo": f(Wo),
        "bq": f(bq), "bk": f(bk), "bv": f(bv), "bo": f(bo),
        "g1": f(g1), "b1": f(b1), "g2": f(g2), "b2": f(b2),
    }
    X = np.asarray(X, dtype=np.float32)
    Y = np.asarray(Y, dtype=np.float32)
    mask = np.asarray(mask)
    in_maps = []
    for b in range(8):
        mb = np.where(mask[b], np.float32(-1e4), np.float32(0.0)).astype(np.float32)
        in_maps.append({
            "XT": np.ascontiguousarray(X[b].T),
            "YT": np.ascontiguousarray(Y[b].T),
            "MB": mb,
            **shared,
        })
    return in_maps


def kernel(X, Y, mask, Wq, bq, Wk, bk, Wv, bv, Wo, bo, g1, b1, g2, b2,
           _trace=False):
    nc = _get_nc()
    in_maps = make_in_maps(X, Y, mask, Wq, bq, Wk, bk, Wv, bv, Wo, bo,
                           g1, b1, g2, b2)
    res = run_bass_kernel_spmd(nc, in_maps, core_ids=list(range(8)),
                               trace=_trace)
    out = np.stack([np.ascontiguousarray(res.results[b]["OT"].T)
                    for b in range(8)]).astype(np.float32)
    if _trace:
        return out, res
    return out


# revision 21
# speedup vs baseline: 1.1377x; 1.1377x over previous
"""Trainium2 Bass kernel for a masked-attention block (MAB).

Computation (per batch element, all fp32):
    Q = X@Wq + bq ; K = Y@Wk + bk ; V = Y@Wv + bv
    logits = per-head Qh@Kh^T / 32, masked keys -> -inf, softmax over keys
    attn   = A @ Vh (concat heads)
    O1 = LN(Q + attn; g1,b1)
    O  = LN(O1 + relu(O1@Wo + bo); g2,b2)

Sharding: pure data-parallel, one batch element per NeuronCore (B=8 = 8 cores).

On-device dataflow is "feature-major": activations live in SBUF transposed
([model_dim -> 8x128 partitions, token -> free]).  With weights in natural
layout every matmul chains without any transposes:
    actT_out[n, t] = sum_d W[d, n] * actT_in[d, t]   (lhsT=W, rhs=actT_in)
Attention also chains: logitsT[k, q] from (lhsT=KT_h, rhs=QT_h) single
128-contraction; exp on ACT (mask folded in as a per-partition bias);
AV from (lhsT=V_natural, rhs=expT).  The softmax denominator and the
LayerNorm stats are partition-dim reductions done with all-ones stationary
matmuls (which also broadcast the result across partitions for free).
All matmuls use float32r (FP22 truncation) which runs at full PE rate for
moving free-dim >= 256.

The host transposes X/Y on the way in and the output on the way out, and
converts the bool mask into an additive f32 bias (0 / -1e4).
"""

import math
import numpy as np
from contextlib import ExitStack

import concourse.bass as bass
import concourse.mybir as mybir
import concourse.tile as tile
from concourse import bacc
from concourse.bass_utils import run_bass_kernel_spmd

P = 128
NX = 1024
NY = 1024
DIM = 1024
H = 8
KO = DIM // P          # 8 partition sub-tiles of the model dim
QC = 512               # moving-operand chunk (fp32 max free dim)
NQC = NX // QC         # 2
F32 = mybir.dt.float32
F32R = mybir.dt.float32r
AF = mybir.ActivationFunctionType
ALU = mybir.AluOpType
SCALE = 1.0 / 32.0     # 1/sqrt(DIM)
EPS = 1e-5


def _r(ap):
    return ap.bitcast(F32R)


def _build():
    nc = bacc.Bacc("TRN2", target_bir_lowering=False, debug=False,
                   enable_asserts=False)

    # ---- DRAM I/O (per-core shapes) ----
    XT = nc.dram_tensor("XT", [DIM, NX], F32, kind="ExternalInput").ap()
    YT = nc.dram_tensor("YT", [DIM, NY], F32, kind="ExternalInput").ap()
    MB = nc.dram_tensor("MB", [NY], F32, kind="ExternalInput").ap()
    Wd = {}
    for w in ("Wq", "Wk", "Wv", "Wo"):
        Wd[w] = nc.dram_tensor(w, [DIM, DIM], F32, kind="ExternalInput").ap()
    Vecs = {}
    for vname in ("bq", "bk", "bv", "bo", "g1", "b1", "g2", "b2"):
        Vecs[vname] = nc.dram_tensor(vname, [DIM], F32, kind="ExternalInput").ap()
    OT = nc.dram_tensor("OT", [DIM, NX], F32, kind="ExternalOutput").ap()

    xt3 = XT.rearrange("(ko p) q -> p ko q", p=P)
    yt3 = YT.rearrange("(ko p) q -> p ko q", p=P)
    wq3 = Wd["Wq"].rearrange("(ko p) d -> p ko d", p=P)
    wk3 = Wd["Wk"].rearrange("(ko p) d -> p ko d", p=P)
    wv3 = Wd["Wv"].rearrange("(ko p) d -> p ko d", p=P)
    wo3 = Wd["Wo"].rearrange("(ko p) d -> p ko d", p=P)
    ot3 = OT.rearrange("(do p) q -> p do q", p=P)

    with tile.TileContext(nc) as tc:
        with ExitStack() as octx:
            const = octx.enter_context(tc.tile_pool(name="const", bufs=1))
            actp = octx.enter_context(tc.tile_pool(name="act", bufs=3))

            # ---- constants ----
            # walrus requires every writer of an fp32r-matmul operand to have
            # an fp32r-tagged output AP; memset can't write f32r, so round
            # the ones through a copy
            ones128 = const.tile([P, P], F32, tag="ones", name="ones128")
            ones_tmp = const.tile([P, P], F32, tag="onest", name="ones_tmp")
            nc.vector.memset(ones_tmp, 1.0)
            nc.vector.tensor_copy(_r(ones128), ones_tmp)
            eps_sb = const.tile([P, 1], F32, tag="eps", name="eps_sb")
            nc.vector.memset(eps_sb, EPS)

            def vec_pko(name):
                t = const.tile([P, KO], F32, tag=f"v_{name}", name=f"{name}_sb")
                nc.sync.dma_start(t, Vecs[name].rearrange("(ko p) -> p ko", p=P))
                return t

            mb_sb = const.tile([P, KO], F32, tag="v_mb", name="mb_sb")
            nc.sync.dma_start(mb_sb, MB.rearrange("(ko p) -> p ko", p=P))
            bq_sb = vec_pko("bq")
            bk_sb = vec_pko("bk")
            bo_sb = vec_pko("bo")
            g1_sb = vec_pko("g1")
            b1_sb = vec_pko("b1")
            g2_sb = vec_pko("g2")
            b2_sb = vec_pko("b2")
            bv_sb = const.tile([1, DIM], F32, tag="v_bv", name="bv_sb")
            nc.sync.dma_start(_r(bv_sb),
                              _r(Vecs["bv"].rearrange("(one n) -> one n", one=1)))

            # ---- big feature-major activation tiles (rotating slots) ----
            qt = actp.tile([P, KO, NX], F32, tag="big", name="qt")
            ktm = actp.tile([P, KO, NY], F32, tag="big", name="ktm")
            vm = actp.tile([P, KO, DIM], F32, tag="big", name="vm")

            # ================= Phase 1: Q, K, V projections =================
            with tc.tile_pool(name="io", bufs=1) as iop, \
                 tc.tile_pool(name="w1", bufs=2) as wp, \
                 tc.tile_pool(name="gp1", bufs=8, space="PSUM") as pp:
                xt = iop.tile([P, KO, NX], F32, tag="xt", name="xt")
                yt = iop.tile([P, KO, NY], F32, tag="yt", name="yt")
                for k in range(KO):
                    nc.sync.dma_start(_r(xt[:, k, :]), _r(xt3[:, k, :]))
                for k in range(KO):
                    nc.sync.dma_start(_r(yt[:, k, :]), _r(yt3[:, k, :]))

                def proj_featmajor(w3, rhs_sb, out_sb, bias_sb, label):
                    # out_sb[p, do, q] (+= bias[do*128+p]) = sum_k W[k, d] rhs[k, q]
                    for dg in range(2):
                        wt = wp.tile([P, KO, QC], F32, tag="w", name=f"w_{label}{dg}")
                        for k in range(KO):
                            nc.sync.dma_start(_r(wt[:, k, :]),
                                              _r(w3[:, k, dg * QC:(dg + 1) * QC]))
                        for qc in range(NQC):
                            qs = slice(qc * QC, (qc + 1) * QC)
                            for d4 in range(4):
                                ps = pp.tile([P, QC], F32, tag="ps",
                                             name=f"ps_{label}{dg}{qc}{d4}")
                                for k in range(KO):
                                    nc.tensor.matmul(
                                        ps,
                                        lhsT=_r(wt[:, k, d4 * P:(d4 + 1) * P]),
                                        rhs=_r(rhs_sb[:, k, qs]),
                                        start=(k == 0), stop=(k == KO - 1))
                                do = dg * 4 + d4
                                nc.scalar.activation(
                                    _r(out_sb[:, do, qs]), ps, AF.Identity,
                                    bias=bias_sb[:, do:do + 1], scale=1.0)

                proj_featmajor(wq3, xt, qt, bq_sb, "q")
                proj_featmajor(wk3, yt, ktm, bk_sb, "k")

                # V in natural (token-major) layout: V[y, n] = sum_k Y[y,k] Wv[k,n]
                for ng in range(2):
                    wt = wp.tile([P, KO, QC], F32, tag="w", name=f"w_v{ng}")
                    for k in range(KO):
                        nc.sync.dma_start(_r(wt[:, k, :]),
                                          _r(wv3[:, k, ng * QC:(ng + 1) * QC]))
                    ns = slice(ng * QC, (ng + 1) * QC)
                    for yo in range(KO):
                        ps = pp.tile([P, QC], F32, tag="ps", name=f"ps_v{ng}{yo}")
                        for k in range(KO):
                            nc.tensor.matmul(
                                ps,
                                lhsT=_r(yt[:, k, yo * P:(yo + 1) * P]),
                                rhs=_r(wt[:, k, :]),
                                start=(k == 0), stop=False)
                        # fold per-free-dim bias bv with a K=1 ones matmul
                        nc.tensor.matmul(
                            ps, lhsT=_r(ones128[0:1, :]), rhs=_r(bv_sb[:, ns]),
                            start=False, stop=True)
                        nc.scalar.copy(_r(vm[:, yo, ns]), ps)

            # ================= Phase 2: attention =================
            with tc.tile_pool(name="zp", bufs=1) as zp:
                zt = zp.tile([P, KO, NX], F32, tag="z", name="zt")

                with tc.tile_pool(name="exp", bufs=12) as ep, \
                     tc.tile_pool(name="rcp", bufs=2) as rp, \
                     tc.tile_pool(name="lgp", bufs=2, space="PSUM") as lgp, \
                     tc.tile_pool(name="avp", bufs=1, space="PSUM") as avp, \
                     tc.tile_pool(name="rlp", bufs=1, space="PSUM") as rlp:
                    for h in range(H):
                        et = [ep.tile([P, NY], F32, tag="exp", name=f"et{h}_{k}")
                              for k in range(KO)]
                        # logitsT[k, q] = sum_d KT_h[d, k] QT_h[d, q]; exp with
                        # mask bias per key (partition) and 1/32 scale.  The
                        # logits psum tile spans 2 banks so one ACT op covers
                        # the whole [128, 1024] key-slice.
                        for kt in range(KO):
                            pl = lgp.tile([P, NX], F32, tag="lg",
                                          name=f"pl{h}{kt}")
                            for qc in range(NQC):
                                qs = slice(qc * QC, (qc + 1) * QC)
                                nc.tensor.matmul(
                                    pl[:, qs],
                                    lhsT=_r(ktm[:, h, kt * P:(kt + 1) * P]),
                                    rhs=_r(qt[:, h, qs]),
                                    start=True, stop=True)
                            nc.scalar.activation(
                                _r(et[kt]), pl, AF.Exp,
                                bias=mb_sb[:, kt:kt + 1], scale=SCALE)
                        # softmax denominator: accumulate the all-ones matmul
                        # over the 8 key sub-tiles -> partition-reduction AND
                        # broadcast in one shot (also keeps PE warm here)
                        pr = rlp.tile([P, NX], F32, tag="rl", name=f"pr{h}")
                        for kt in range(KO):
                            for qc in range(NQC):
                                qs = slice(qc * QC, (qc + 1) * QC)
                                nc.tensor.matmul(
                                    pr[:, qs], lhsT=_r(ones128),
                                    rhs=_r(et[kt][:, qs]),
                                    start=(kt == 0), stop=(kt == KO - 1))
                        rc = rp.tile([P, NX], F32, tag="rc", name=f"rc{h}")
                        nc.vector.reciprocal_approx_fast(rc, pr)
                        # attnT_h[d, q] = sum_k V[k, d_h] expT[k, q]; then
                        # normalize by the softmax denom and add the Q residual
                        pa = avp.tile([P, NX], F32, tag="av", name=f"pa{h}")
                        for qc in range(NQC):
                            qs = slice(qc * QC, (qc + 1) * QC)
                            for kt in range(KO):
                                nc.tensor.matmul(
                                    pa[:, qs],
                                    lhsT=_r(vm[:, kt, h * P:(h + 1) * P]),
                                    rhs=_r(et[kt][:, qs]),
                                    start=(kt == 0), stop=(kt == KO - 1))
                        nc.vector.tensor_mul(_r(zt[:, h, :]), pa, rc)
                        nc.vector.tensor_add(_r(zt[:, h, :]), zt[:, h, :],
                                             qt[:, h, :])

                # ---- LayerNorm over the model dim (partition direction) ----
                def layernorm(in_sb, sqp, stp, spp, emit_out):
                    for qc in range(NQC):
                        qs = slice(qc * QC, (qc + 1) * QC)
                        pmu = spp.tile([P, QC], F32, tag="pmu", name=f"pmu{qc}")
                        ps2 = spp.tile([P, QC], F32, tag="ps2", name=f"ps2{qc}")
                        for do in range(KO):
                            nc.tensor.matmul(pmu, lhsT=_r(ones128),
                                             rhs=_r(in_sb[:, do, qs]),
                                             start=(do == 0), stop=(do == KO - 1))
                        for do in range(KO):
                            sq = sqp.tile([P, QC], F32, tag="sq", name=f"sq{qc}{do}")
                            nc.vector.tensor_mul(_r(sq), in_sb[:, do, qs],
                                                 in_sb[:, do, qs])
                            nc.tensor.matmul(ps2, lhsT=_r(ones128), rhs=_r(sq),
                                             start=(do == 0), stop=(do == KO - 1))
                        mu = stp.tile([P, QC], F32, tag="mu", name=f"mu{qc}")
                        nc.vector.tensor_scalar_mul(mu, pmu, 1.0 / DIM)
                        msq = stp.tile([P, QC], F32, tag="msq", name=f"msq{qc}")
                        nc.vector.tensor_mul(msq, mu, mu)
                        sd = stp.tile([P, QC], F32, tag="sd", name=f"sd{qc}")
                        nc.vector.scalar_tensor_tensor(
                            sd, ps2, 1.0 / DIM, msq,
                            op0=ALU.mult, op1=ALU.subtract)
                        nc.scalar.activation(sd, sd, AF.Sqrt, bias=eps_sb, scale=1.0)
                        rsig = stp.tile([P, QC], F32, tag="rsig", name=f"rsig{qc}")
                        nc.vector.reciprocal_approx_fast(rsig, sd)
                        mrs = stp.tile([P, QC], F32, tag="mrs", name=f"mrs{qc}")
                        nc.vector.tensor_mul(mrs, mu, rsig)
                        for do in range(KO):
                            t = sqp.tile([P, QC], F32, tag="t", name=f"t{qc}{do}")
                            nc.vector.tensor_mul(t, in_sb[:, do, qs], rsig)
                            nc.vector.tensor_sub(t, t, mrs)
                            emit_out(do, qs, t)

                # LN1 -> o1t (feature-major)
                with tc.tile_pool(name="sq1", bufs=3) as sqp1, \
                     tc.tile_pool(name="st1", bufs=2) as stp1, \
                     tc.tile_pool(name="sp1", bufs=2, space="PSUM") as spp1:
                    o1t = actp.tile([P, KO, NX], F32, tag="big", name="o1t")

                    def emit_o1(do, qs, t):
                        nc.vector.tensor_scalar(
                            _r(o1t[:, do, qs]), t,
                            scalar1=g1_sb[:, do:do + 1],
                            scalar2=b1_sb[:, do:do + 1],
                            op0=ALU.mult, op1=ALU.add)

                    layernorm(zt, sqp1, stp1, spp1, emit_o1)

            # ================= Phase 3: output proj + LN2 =================
            with tc.tile_pool(name="w3", bufs=2) as wp3, \
                 tc.tile_pool(name="sq2", bufs=4) as sqp2, \
                 tc.tile_pool(name="st2", bufs=2) as stp2, \
                 tc.tile_pool(name="out", bufs=4) as outp, \
                 tc.tile_pool(name="gp3", bufs=4, space="PSUM") as pp3, \
                 tc.tile_pool(name="sp2", bufs=2, space="PSUM") as spp2:
                z2t = actp.tile([P, KO, NX], F32, tag="big", name="z2t")
                # HT[n, q] = sum_d Wo[d, n] O1T[d, q];  z2 = o1 + relu(H + bo)
                for ng in range(2):
                    wt = wp3.tile([P, KO, QC], F32, tag="w", name=f"w_o{ng}")
                    for k in range(KO):
                        nc.sync.dma_start(_r(wt[:, k, :]),
                                          _r(wo3[:, k, ng * QC:(ng + 1) * QC]))
                    for qc in range(NQC):
                        qs = slice(qc * QC, (qc + 1) * QC)
                        for n4 in range(4):
                            ps = pp3.tile([P, QC], F32, tag="ps",
                                          name=f"ps_o{ng}{qc}{n4}")
                            for k in range(KO):
                                nc.tensor.matmul(
                                    ps,
                                    lhsT=_r(wt[:, k, n4 * P:(n4 + 1) * P]),
                                    rhs=_r(o1t[:, k, qs]),
                                    start=(k == 0), stop=(k == KO - 1))
                            no = ng * 4 + n4
                            ht = sqp2.tile([P, QC], F32, tag="ht",
                                           name=f"ht{ng}{qc}{n4}")
                            nc.scalar.activation(ht, ps, AF.Relu,
                                                 bias=bo_sb[:, no:no + 1], scale=1.0)
                            nc.vector.tensor_add(_r(z2t[:, no, qs]), ht,
                                                 o1t[:, no, qs])

                def emit_o2(do, qs, t):
                    o = outp.tile([P, QC], F32, tag="o", name=f"o{do}")
                    nc.vector.tensor_scalar(
                        o, t,
                        scalar1=g2_sb[:, do:do + 1],
                        scalar2=b2_sb[:, do:do + 1],
                        op0=ALU.mult, op1=ALU.add)
                    nc.sync.dma_start(ot3[:, do, qs], o)

                layernorm(z2t, sqp2, stp2, spp2, emit_o2)

    nc.compile()
    return nc


_CACHE = {}


def _get_nc():
    if "nc" not in _CACHE:
        _CACHE["nc"] = _build()
    return _CACHE["nc"]


def make_in_maps(X, Y, mask, Wq, bq, Wk, bk, Wv, bv, Wo, bo, g1, b1, g2, b2):
    f = lambda a: np.ascontiguousarray(np.asarray(a, dtype=np.float32))
    shared = {
        "Wq": f(Wq), "Wk": f(Wk), "Wv": f(Wv), "Wo": f(Wo),
        "bq": f(bq), "bk": f(bk), "bv": f(bv), "bo": f(bo),
        "g1": f(g1), "b1": f(b1), "g2": f(g2), "b2": f(b2),
    }
    X = np.asarray(X, dtype=np.float32)
    Y = np.asarray(Y, dtype=np.float32)
    mask = np.asarray(mask)
    in_maps = []
    for b in range(8):
        mb = np.where(mask[b], np.float32(-1e4), np.float32(0.0)).astype(np.float32)
        in_maps.append({
            "XT": np.ascontiguousarray(X[b].T),
            "YT": np.ascontiguousarray(Y[b].T),
            "MB": mb,
            **shared,
        })
    return in_maps


def kernel(X, Y, mask, Wq, bq, Wk, bk, Wv, bv, Wo, bo, g1, b1, g2, b2,
           _trace=False):
    nc = _get_nc()
    in_maps = make_in_maps(X, Y, mask, Wq, bq, Wk, bk, Wv, bv, Wo, bo,
                           g1, b1, g2, b2)
    res = run_bass_kernel_spmd(nc, in_maps, core_ids=list(range(8)),
                               trace=_trace)
    out = np.stack([np.ascontiguousarray(res.results[b]["OT"].T)
                    for b in range(8)]).astype(np.float32)
    if _trace:
        return out, res
    return out
